# revision 38
# baseline (speedup 1.0000x reference)
"""HGNN conv kernel for Trainium2, data-parallel over time across 8 cores.

Per core (t = core index): out_b = Dv^-1/2 Gc De^-1 Gc^T Dv^-1/2 (x_b W + 1 b^T)
computed in factored form (L never materialized):
  Gs  = Dv^-1/2 Gc                       [N, E]   (bf16)
  z   = x^T Gs per 128-col bf block      [BF, E]  (MM1)
  zw  = z^T-blocks @ blockdiag(W,W)      [E, BF]  (W-MM; + u0 (x) bias add)
  out = Gsd^T v with Gsd = de * Gs^T     [N, BF]  (MM2)

All matmul operands are bf16 (fp32-family moving operands stream at ~2.4
cycles/col on TRN2 PE vs 1 for bf16).  Nodes are swizzled n = p*8 + n2 so one
SBUF partition covers 8 consecutive DRAM rows -> 2KB-contiguous HBM
descriptors for x loads (f32 staging + permute-cast spread over DVE/ACT/
GpSimd) and for out stores (SBUF staging).  All input DMA rides one HWDGE
FIFO queue: Gc chunks first (they gate the whole stats pipeline), then x
chunks, so MM1 starts ~14us in and is PE-bound after.  MM2 runs on bf column
chunks interleaved into the m loop so output DMA overlaps compute; tail
chunks shrink so the post-loop drain is short.  Dummy matmuls at t=0 warm
the PE HAM clock to 2.4 GHz.
"""

import sys

import numpy as np

sys.path.insert(0, "/opt/trn_rl_repo")

from contextlib import ExitStack

import concourse.bass as bass
import concourse.mybir as mybir
import concourse.tile as tile
from concourse import bacc, bass_utils
from concourse.masks import make_identity

P = 128
T = 8
B = 28          # batch entries per core
N = 1024        # nodes
E = 512         # hyperedges (256 static + 256 dynamic)
F = 64          # features
BF = B * F      # 1792
EPS = 1e-6
N2 = 8          # node swizzle: n = p*8 + n2
MT = BF // P    # 14 bf-tiles (2 batch entries each)
ET = E // P     # 4 e-tiles
# MM2 output column chunks (bf columns); tail chunks shrink so the last
# MM2+evict+store after the final m-tile is short
CHUNKS = [(0, 512), (512, 1024), (1024, 1408), (1408, 1664), (1664, 1792)]
# emit chunk c right after wmm(m) for m = CHUNK_AFTER[c]
CHUNK_AFTER = {4: 0, 8: 1, 11: 2, 12: 3, 13: 4}

f32 = mybir.dt.float32
f32r = mybir.dt.float32r
bf16 = mybir.dt.bfloat16


def _build_nc():
    nc = bacc.Bacc("TRN2", target_bir_lowering=False, debug=False)

    xs = nc.dram_tensor("xs", [B, N, F], f32, kind="ExternalInput").ap()
    g = nc.dram_tensor("g", [N, 256], f32r, kind="ExternalInput").ap()
    g1 = nc.dram_tensor("g1", [N, 256], f32r, kind="ExternalInput").ap()
    w = nc.dram_tensor("w", [F, F], f32, kind="ExternalInput").ap()
    bvec = nc.dram_tensor("b", [F], f32, kind="ExternalInput").ap()
    os_ = nc.dram_tensor("os", [B, N, F], f32, kind="ExternalOutput").ap()

    with tile.TileContext(nc) as tc, ExitStack() as ctx:
        const = ctx.enter_context(tc.tile_pool(name="const", bufs=1))
        big = ctx.enter_context(tc.tile_pool(name="big", bufs=1))
        xstage = ctx.enter_context(tc.tile_pool(name="xstage", bufs=6))
        ztp = ctx.enter_context(tc.tile_pool(name="ztp", bufs=3))
        osb = ctx.enter_context(tc.tile_pool(name="osb", bufs=2))
        # PSUM: 8 banks, bank-granular per site x bufs:
        # zps 2 + stats 1 + wps(+warm) 1 + ops 2 + transposes 2 = 8
        ps_z = ctx.enter_context(tc.tile_pool(name="ps_z", bufs=2, space="PSUM"))
        ps_st = ctx.enter_context(tc.tile_pool(name="ps_st", bufs=1, space="PSUM"))
        ps_w = ctx.enter_context(tc.tile_pool(name="ps_w", bufs=1, space="PSUM"))
        ps_o = ctx.enter_context(tc.tile_pool(name="ps_o", bufs=2, space="PSUM"))
        ps_t = ctx.enter_context(tc.tile_pool(name="ps_t", bufs=2, space="PSUM"))

        # ---- input DMA: one HWDGE FIFO (sync queue) ----------------------
        bdw_f = const.tile([P, P], f32, name="bdw_f")
        nc.vector.memset(bdw_f[:], 0.0)
        nc.sync.dma_start(bdw_f[0:64, 0:64], w)
        nc.sync.dma_start(bdw_f[64:128, 64:128], w)
        btmp = const.tile([1, F], f32, name="btmp")
        nc.sync.dma_start(btmp[:], bvec[None, :])

        # G and G1 in separate [p, n2, 256] tiles with n = p*8 + n2: both
        # sides of each DMA are fully contiguous per partition -> 8KB
        # descriptors (vs 1KB for an interleaved [G|G1] tile); halves so the
        # per-k stats pipeline starts early
        gcg = big.tile([P, N2, 256], f32r, name="gcg")
        gcd = big.tile([P, N2, 256], f32r, name="gcd")
        g_r = g.rearrange("(p n2) e -> p n2 e", p=P)
        g1_r = g1.rearrange("(p n2) e -> p n2 e", p=P)
        # x staging: [p, b, n2, f] f32, 2KB-contiguous on both sides.
        # One serial HWDGE FIFO for all input (concurrent rings measured at
        # ~92 GB/s each vs ~246 for a single busy ring).  Order: x0, x1
        # (their casts overlap the gc transfer), gc, then x2..13.
        xs_r = xs.rearrange("b (p n2) f -> p b n2 f", p=P)
        xstages = {}

        def stage_x(m):
            xf = xstage.tile([P, 2, N2, F], f32, name="xf")
            nc.sync.dma_start(xf[:], xs_r[:, 2 * m : 2 * m + 2])
            xstages[m] = xf

        stage_x(0)
        stage_x(1)
        for h in range(2):
            nc.sync.dma_start(gcg[:, 4 * h : 4 * h + 4], g_r[:, 4 * h : 4 * h + 4])
            nc.sync.dma_start(gcd[:, 4 * h : 4 * h + 4], g1_r[:, 4 * h : 4 * h + 4])
        for m in range(2, MT):
            stage_x(m)

        # permute-cast staged x -> [p, n2, b, f] bf16 (MM1 lhsT layout),
        # two half-chunks per m spread across DVE / ACT / GpSimd
        xs_all = big.tile([P, N2, B, F], bf16, name="xs_all")

        def cast_half(m, h):
            dst = xs_all[:, 4 * h : 4 * h + 4, 2 * m : 2 * m + 2, :]
            src = xstages[m][:, :, 4 * h : 4 * h + 4, :].rearrange(
                "p b k f -> p k b f"
            )
            if m <= 3:
                (nc.vector.tensor_copy if h == 0 else nc.scalar.copy)(dst, src)
            else:
                (nc.gpsimd.tensor_copy if h == 0 else nc.vector.tensor_copy)(dst, src)

        # ---- constants / PE warmup --------------------------------------
        ident_f = const.tile([P, P], f32, name="ident_f")
        make_identity(nc, ident_f[:])
        ident_b = const.tile([P, P], bf16, name="ident_b")
        nc.vector.tensor_copy(ident_b[:], ident_f[:])

        # PE warmup gated on the first gc chunk so it lands right before the
        # stats matmuls -- early enough to flip HAM to 2.4 GHz, late enough
        # that the window doesn't re-throttle before the dense phase
        warm_sb = const.tile([P, 512], bf16, name="warm_sb")
        nc.vector.tensor_copy(warm_sb[:], gcg[:, 0:2, :])
        with tc.high_priority():
            warm_ps = ps_w.tile([P, 512], f32, name="wps")
            for i in range(6):
                nc.tensor.matmul(
                    warm_ps[:], ident_b[:], warm_sb[:], start=(i == 0), stop=(i == 5)
                )

        bdw = const.tile([P, P], bf16, name="bdw")
        nc.vector.tensor_copy(bdw[:], bdw_f[:])

        bias2 = const.tile([1, 2, F], f32, name="bias2")
        nc.vector.tensor_copy(bias2[:], btmp[0:1, None, :].to_broadcast([1, 2, F]))
        bias_bc = const.tile([P, P], f32, name="bias_bc")
        nc.gpsimd.partition_broadcast(
            bias_bc[:], bias2[:].rearrange("o t f -> o (t f)")
        )

        # ---- degree stats, pipelined per n2 ------------------------------
        # dv = 1/sqrt(rowsum(Gc) + eps); gs = dv * Gc (bf16) per k as Gc
        # chunks land
        rsg = const.tile([P, N2], f32, name="rsg")
        rs = const.tile([P, N2], f32, name="rs")
        rs_junk = const.tile([P, 256], f32, name="rs_junk")
        sq = const.tile([P, N2], f32, name="sq")
        dv = const.tile([P, N2], f32, name="dv")
        gs_all = big.tile([P, N2, E], bf16, name="gs_all")
        # lhsT per n2: [sq | ones] against gs -> colsums of Gc (row 0,
        # since sq*gs = Gc) and of Gs (row 1) in one bf16 matmul chain
        onesq = const.tile([P, N2, 2], bf16, name="onesq")
        nc.vector.memset(onesq[:, :, 1:2], 1.0)
        stats_ps = ps_st.tile([2, E], f32, name="stats_ps")
        for k in range(N2):
            # rowsum(Gc) = rowsum(G) + rowsum(G1), the halves folded via the
            # sqrt bias; eps (1e-6) is negligible against rowsums ~256
            nc.vector.reduce_sum(
                rsg[:, k : k + 1], gcg[:, k, :], axis=mybir.AxisListType.X
            )
            nc.scalar.activation(
                rs_junk[:], gcd[:, k, :],
                mybir.ActivationFunctionType.Copy,
                accum_out=rs[:, k : k + 1],
            )
            nc.scalar.activation(
                sq[:, k : k + 1], rs[:, k : k + 1],
                mybir.ActivationFunctionType.Sqrt, bias=rsg[:, k : k + 1],
            )
            nc.vector.reciprocal(dv[:, k : k + 1], sq[:, k : k + 1])
            nc.gpsimd.tensor_copy(onesq[:, k, 0:1], sq[:, k : k + 1])
            nc.vector.tensor_scalar(
                out=gs_all[:, k, 0:256], in0=gcg[:, k, :], scalar1=dv[:, k : k + 1],
                scalar2=None, op0=mybir.AluOpType.mult,
            )
            nc.scalar.activation(
                gs_all[:, k, 256:512], gcd[:, k, :],
                mybir.ActivationFunctionType.Copy, scale=dv[:, k : k + 1],
            )
            nc.tensor.matmul(
                stats_ps[:], onesq[:, k, :], gs_all[:, k, :],
                start=(k == 0), stop=(k == N2 - 1),
            )
        stats_sb = const.tile([2, E], bf16, name="stats_sb")
        nc.vector.tensor_copy(stats_sb[:], stats_ps[:])

        # transpose stats to column layout [128, ET, 2] = [cs | u0]
        statsT = const.tile([P, ET, 2], f32, name="statsT")
        for j in range(ET):
            tp = ps_t.tile([P, P], bf16, name="sp")[:, 0:2]
            nc.tensor.matmul(
                tp[:], stats_sb[:, j * P : (j + 1) * P], ident_b[0:2, 0:2],
                is_transpose=True,
            )
            nc.vector.tensor_copy(statsT[:, j, :], tp[:])
        de_col = const.tile([P, ET], f32, name="de_col")
        nc.vector.tensor_scalar(
            out=de_col[:], in0=statsT[:, :, 0], scalar1=EPS, scalar2=None,
            op0=mybir.AluOpType.add,
        )
        nc.vector.reciprocal(de_col[:], de_col[:])

        # ub[e-part, j, bf2] = u0[e] * bias-pattern  (added to every zw m-tile)
        ub = const.tile([P, ET, P], f32, name="ub")
        for j in range(ET):
            nc.vector.tensor_scalar(
                out=ub[:, j, :], in0=bias_bc[:], scalar1=statsT[:, j, 1:2],
                scalar2=None, op0=mybir.AluOpType.mult,
            )

        # Gsd[e, n-col] = de[e] * Gs[n, e] via PE transpose + scaled evict
        # n-col order is (n2, q): col n2*128+q holds n = q*8 + n2
        gsd_all = big.tile([P, ET, N], bf16, name="gsd_all")

        def gsd_strip(k):
            for j in range(ET):
                tp = ps_t.tile([P, P], bf16, name="sp")
                nc.tensor.matmul(
                    tp[:], gs_all[:, k, j * P : (j + 1) * P], ident_b[:],
                    is_transpose=True,
                )
                if (k * ET + j) % 2 == 0:
                    nc.vector.tensor_scalar(
                        out=gsd_all[:, j, k * P : (k + 1) * P], in0=tp[:],
                        scalar1=de_col[:, j : j + 1], scalar2=None,
                        op0=mybir.AluOpType.mult,
                    )
                else:
                    nc.scalar.activation(
                        gsd_all[:, j, k * P : (k + 1) * P], tp[:],
                        mybir.ActivationFunctionType.Copy,
                        scale=de_col[:, j : j + 1],
                    )

        # ---- main pipeline ----------------------------------------------
        # v_all[e-part, j, bf] = zw + u0*bias  (bf16)
        v_all = big.tile([P, ET, BF], bf16, name="v_all")
        os_r = os_.rearrange("b (p n2) f -> p b n2 f", p=P)

        def mm1(m):
            zps = ps_z.tile([P, E], f32, name="zps")
            for k in range(N2):
                nc.tensor.matmul(
                    zps[:], xs_all[:, k, 2 * m : 2 * m + 2, :], gs_all[:, k, :],
                    start=(k == 0), stop=(k == N2 - 1),
                )
            return zps

        def wmm(m, zps):
            zt = ztp.tile([P, E], bf16, name="zt")
            nc.scalar.copy(zt[:], zps[:])
            wps = ps_w.tile([P, E], f32, name="wps")
            for j in range(ET):
                nc.tensor.matmul(
                    wps[:, j * P : (j + 1) * P], zt[:, j * P : (j + 1) * P], bdw[:],
                    start=True, stop=True,
                )
            # v = ub + zw for all 4 j-blocks in one DVE op
            nc.vector.scalar_tensor_tensor(
                out=v_all[:, :, m * P : (m + 1) * P],
                in0=ub[:],
                scalar=1.0,
                in1=wps[:].rearrange("p (j c) -> p j c", j=ET),
                op0=mybir.AluOpType.mult,
                op1=mybir.AluOpType.add,
            )

        def mm2(c):
            c0, c1 = CHUNKS[c]
            nb = (c1 - c0) // F  # batch entries in this chunk
            ob = osb.tile([P, 8, N2, F], f32, name="ob")
            for k in range(N2):
                ops = ps_o.tile([P, 512], f32, name="ops")[:, 0 : c1 - c0]
                for j in range(ET):
                    nc.tensor.matmul(
                        ops[:], gsd_all[:, j, k * P : (k + 1) * P],
                        v_all[:, j, c0:c1],
                        start=(j == 0), stop=(j == ET - 1),
                    )
                dst = ob[:, 0:nb, k, :]
                src = ops[:].rearrange("p (c f) -> p c f", f=F)
                if k % 2 == 0:
                    nc.scalar.copy(dst, src)
                else:
                    nc.vector.tensor_copy(dst, src)
            if c >= 3:
                # tail chunks: store each n2-half as soon as its evicts land
                nc.scalar.dma_start(
                    os_r[:, c0 // F : c1 // F, 0:4], ob[:, 0:nb, 0:4, :]
                )
                nc.scalar.dma_start(
                    os_r[:, c0 // F : c1 // F, 4:8], ob[:, 0:nb, 4:8, :]
                )
            else:
                nc.scalar.dma_start(os_r[:, c0 // F : c1 // F], ob[:, 0:nb, :, :])

        # gsd strips are emitted after mm1(1) so the scheduler doesn't place
        # the 32 transposes ahead of MM1(0) -- they're only needed by the
        # first mm2 chunk
        cast_half(0, 0)
        cast_half(0, 1)
        cast_half(1, 0)
        cast_half(1, 1)
        zps_prev = mm1(0)
        for m in range(1, MT):
            if m + 1 < MT:
                cast_half(m + 1, 0)
                cast_half(m + 1, 1)
            zps = mm1(m)
            if m == 2:
                for k in range(N2):
                    gsd_strip(k)
            wmm(m - 1, zps_prev)
            zps_prev = zps
            if m - 1 in CHUNK_AFTER:
                mm2(CHUNK_AFTER[m - 1])
        wmm(MT - 1, zps_prev)
        mm2(CHUNK_AFTER[MT - 1])

    nc.finalize()
    return nc


_NC = None


def _get_nc():
    global _NC
    if _NC is None:
        _NC = _build_nc()
    return _NC


def kernel(x, G, G1, weight, bias):
    nc = _get_nc()
    x = np.ascontiguousarray(x, dtype=np.float32)
    G = np.ascontiguousarray(G, dtype=np.float32)
    G1 = np.ascontiguousarray(G1, dtype=np.float32)
    weight = np.ascontiguousarray(weight, dtype=np.float32)
    bias = np.ascontiguousarray(bias, dtype=np.float32)

    in_maps = []
    for c in range(T):
        in_maps.append(
            {
                "xs": x[c * B : (c + 1) * B],
                "g": G,
                "g1": np.ascontiguousarray(G1[c]),
                "w": weight,
                "b": bias,
            }
        )
    res = bass_utils.run_bass_kernel_spmd(nc, in_maps, core_ids=list(range(T)))
    return np.concatenate([r["os"] for r in res.results], axis=0)


# revision 44
# speedup vs baseline: 1.0270x; 1.0270x over previous
"""HGNN conv kernel for Trainium2, data-parallel over time across 8 cores.

Per core (t = core index): out_b = Dv^-1/2 Gc De^-1 Gc^T Dv^-1/2 (x_b W + 1 b^T)
computed in factored form (L never materialized):
  Gs  = Dv^-1/2 Gc                       [N, E]   (bf16)
  z   = x^T Gs per 128-col bf block      [BF, E]  (MM1)
  zw  = z^T-blocks @ blockdiag(W,W)      [E, BF]  (W-MM; + u0 (x) bias add)
  out = Gsd^T v with Gsd = de * Gs^T     [N, BF]  (MM2)

All matmul operands are bf16 (fp32-family moving operands stream at ~2.4
cycles/col on TRN2 PE vs 1 for bf16).  Nodes are swizzled n = p*8 + n2 so one
SBUF partition covers 8 consecutive DRAM rows -> 2KB-contiguous HBM
descriptors for x loads (f32 staging + permute-cast spread over DVE/ACT/
GpSimd) and for out stores (SBUF staging).  All input DMA rides one HWDGE
FIFO queue: Gc chunks first (they gate the whole stats pipeline), then x
chunks, so MM1 starts ~14us in and is PE-bound after.  MM2 runs on bf column
chunks interleaved into the m loop so output DMA overlaps compute; tail
chunks shrink so the post-loop drain is short.  Dummy matmuls at t=0 warm
the PE HAM clock to 2.4 GHz.
"""

import sys

import numpy as np

sys.path.insert(0, "/opt/trn_rl_repo")

from contextlib import ExitStack

import concourse.bass as bass
import concourse.mybir as mybir
import concourse.tile as tile
from concourse import bacc, bass_utils
from concourse.masks import make_identity

P = 128
T = 8
B = 28          # batch entries per core
N = 1024        # nodes
E = 512         # hyperedges (256 static + 256 dynamic)
F = 64          # features
BF = B * F      # 1792
EPS = 1e-6
N2 = 8          # node swizzle: n = p*8 + n2
MT = BF // P    # 14 bf-tiles (2 batch entries each)
ET = E // P     # 4 e-tiles
# MM2 output column chunks (bf columns); tail chunks shrink so the last
# MM2+evict+store after the final m-tile is short
CHUNKS = [(0, 512), (512, 1024), (1024, 1408), (1408, 1664), (1664, 1792)]
# emit chunk c right after wmm(m) for m = CHUNK_AFTER[c]
CHUNK_AFTER = {4: 0, 8: 1, 11: 2, 12: 3, 13: 4}

f32 = mybir.dt.float32
f32r = mybir.dt.float32r
bf16 = mybir.dt.bfloat16


def _build_nc():
    nc = bacc.Bacc("TRN2", target_bir_lowering=False, debug=False)

    xs = nc.dram_tensor("xs", [B, N, F], f32, kind="ExternalInput").ap()
    g = nc.dram_tensor("g", [N, 256], f32r, kind="ExternalInput").ap()
    g1 = nc.dram_tensor("g1", [N, 256], f32r, kind="ExternalInput").ap()
    w = nc.dram_tensor("w", [F, F], f32, kind="ExternalInput").ap()
    bvec = nc.dram_tensor("b", [F], f32, kind="ExternalInput").ap()
    os_ = nc.dram_tensor("os", [B, N, F], f32, kind="ExternalOutput").ap()

    with tile.TileContext(nc) as tc, ExitStack() as ctx:
        const = ctx.enter_context(tc.tile_pool(name="const", bufs=1))
        big = ctx.enter_context(tc.tile_pool(name="big", bufs=1))
        xstage = ctx.enter_context(tc.tile_pool(name="xstage", bufs=6))
        ztp = ctx.enter_context(tc.tile_pool(name="ztp", bufs=3))
        osb = ctx.enter_context(tc.tile_pool(name="osb", bufs=2))
        # PSUM: 8 banks, bank-granular per site x bufs.  zps runs 3 deep (the
        # WMM trails MM1 by two m-tiles so its evict has slack); stats rides
        # the zps pool (done before the third zps rotation); warm+wps+ops
        # share one pool.
        ps_z = ctx.enter_context(tc.tile_pool(name="ps_z", bufs=3, space="PSUM"))
        ps_wo = ctx.enter_context(tc.tile_pool(name="ps_wo", bufs=2, space="PSUM"))
        ps_t = ctx.enter_context(tc.tile_pool(name="ps_t", bufs=2, space="PSUM"))

        # ---- input DMA: one HWDGE FIFO (sync queue) ----------------------
        bdw_f = const.tile([P, P], f32, name="bdw_f")
        nc.vector.memset(bdw_f[:], 0.0)
        nc.sync.dma_start(bdw_f[0:64, 0:64], w)
        nc.sync.dma_start(bdw_f[64:128, 64:128], w)
        btmp = const.tile([1, F], f32, name="btmp")
        nc.sync.dma_start(btmp[:], bvec[None, :])

        # G and G1 in separate [p, n2, 256] tiles with n = p*8 + n2: both
        # sides of each DMA are fully contiguous per partition -> 8KB
        # descriptors (vs 1KB for an interleaved [G|G1] tile); halves so the
        # per-k stats pipeline starts early
        gcg = big.tile([P, N2, 256], f32r, name="gcg")
        gcd = big.tile([P, N2, 256], f32r, name="gcd")
        g_r = g.rearrange("(p n2) e -> p n2 e", p=P)
        g1_r = g1.rearrange("(p n2) e -> p n2 e", p=P)
        # x staging: [p, b, n2, f] f32, 2KB-contiguous on both sides.
        # One serial HWDGE FIFO for all input (concurrent rings measured at
        # ~92 GB/s each vs ~246 for a single busy ring).  Order: x0, x1
        # (their casts overlap the gc transfer), gc, then x2..13.
        xs_r = xs.rearrange("b (p n2) f -> p b n2 f", p=P)
        xstages = {}

        def stage_x(m):
            xf = xstage.tile([P, 2, N2, F], f32, name="xf")
            nc.sync.dma_start(xf[:], xs_r[:, 2 * m : 2 * m + 2])
            xstages[m] = xf

        stage_x(0)
        stage_x(1)
        for h in range(2):
            nc.sync.dma_start(gcg[:, 4 * h : 4 * h + 4], g_r[:, 4 * h : 4 * h + 4])
            nc.sync.dma_start(gcd[:, 4 * h : 4 * h + 4], g1_r[:, 4 * h : 4 * h + 4])
        for m in range(2, MT):
            stage_x(m)

        # permute-cast staged x -> [p, n2, b, f] bf16 (MM1 lhsT layout),
        # two half-chunks per m spread across DVE / ACT / GpSimd
        xs_all = big.tile([P, N2, B, F], bf16, name="xs_all")

        def cast_half(m, h):
            dst = xs_all[:, 4 * h : 4 * h + 4, 2 * m : 2 * m + 2, :]
            src = xstages[m][:, :, 4 * h : 4 * h + 4, :].rearrange(
                "p b k f -> p k b f"
            )
            if m <= 3:
                (nc.vector.tensor_copy if h == 0 else nc.scalar.copy)(dst, src)
            else:
                (nc.gpsimd.tensor_copy if h == 0 else nc.scalar.copy)(dst, src)

        # ---- constants / PE warmup --------------------------------------
        ident_f = const.tile([P, P], f32, name="ident_f")
        make_identity(nc, ident_f[:])
        ident_b = const.tile([P, P], bf16, name="ident_b")
        nc.vector.tensor_copy(ident_b[:], ident_f[:])

        # PE warmup gated on the first gc chunk so it lands right before the
        # stats matmuls -- early enough to flip HAM to 2.4 GHz, late enough
        # that the window doesn't re-throttle before the dense phase
        warm_sb = const.tile([P, 512], bf16, name="warm_sb")
        nc.vector.tensor_copy(warm_sb[:], gcg[:, 0:2, :])
        with tc.high_priority():
            warm_ps = ps_wo.tile([P, 512], f32, name="wps")
            for i in range(6):
                nc.tensor.matmul(
                    warm_ps[:], ident_b[:], warm_sb[:], start=(i == 0), stop=(i == 5)
                )

        bdw = const.tile([P, P], bf16, name="bdw")
        nc.vector.tensor_copy(bdw[:], bdw_f[:])

        bias2 = const.tile([1, 2, F], f32, name="bias2")
        nc.vector.tensor_copy(bias2[:], btmp[0:1, None, :].to_broadcast([1, 2, F]))
        bias_bc = const.tile([P, P], f32, name="bias_bc")
        nc.gpsimd.partition_broadcast(
            bias_bc[:], bias2[:].rearrange("o t f -> o (t f)")
        )

        # ---- degree stats, pipelined per n2 ------------------------------
        # dv = 1/sqrt(rowsum(Gc) + eps); gs = dv * Gc (bf16) per k as Gc
        # chunks land
        rsg = const.tile([P, N2], f32, name="rsg")
        rs = const.tile([P, N2], f32, name="rs")
        rs_junk = const.tile([P, 256], f32, name="rs_junk")
        sq = const.tile([P, N2], f32, name="sq")
        dv = const.tile([P, N2], f32, name="dv")
        gs_all = big.tile([P, N2, E], bf16, name="gs_all")
        # lhsT per n2: [sq | ones] against gs -> colsums of Gc (row 0,
        # since sq*gs = Gc) and of Gs (row 1) in one bf16 matmul chain
        onesq = const.tile([P, N2, 2], bf16, name="onesq")
        nc.vector.memset(onesq[:, :, 1:2], 1.0)
        stats_ps = ps_z.tile([2, E], f32, name="zps")
        for k in range(N2):
            # rowsum(Gc) = rowsum(G) + rowsum(G1), the halves folded via the
            # sqrt bias; eps (1e-6) is negligible against rowsums ~256
            nc.vector.reduce_sum(
                rsg[:, k : k + 1], gcg[:, k, :], axis=mybir.AxisListType.X
            )
            nc.scalar.activation(
                rs_junk[:], gcd[:, k, :],
                mybir.ActivationFunctionType.Copy,
                accum_out=rs[:, k : k + 1],
            )
            nc.scalar.activation(
                sq[:, k : k + 1], rs[:, k : k + 1],
                mybir.ActivationFunctionType.Sqrt, bias=rsg[:, k : k + 1],
            )
            nc.vector.reciprocal(dv[:, k : k + 1], sq[:, k : k + 1])
            nc.gpsimd.tensor_copy(onesq[:, k, 0:1], sq[:, k : k + 1])
            nc.vector.tensor_scalar(
                out=gs_all[:, k, 0:256], in0=gcg[:, k, :], scalar1=dv[:, k : k + 1],
                scalar2=None, op0=mybir.AluOpType.mult,
            )
            nc.scalar.activation(
                gs_all[:, k, 256:512], gcd[:, k, :],
                mybir.ActivationFunctionType.Copy, scale=dv[:, k : k + 1],
            )
            nc.tensor.matmul(
                stats_ps[:], onesq[:, k, :], gs_all[:, k, :],
                start=(k == 0), stop=(k == N2 - 1),
            )
        stats_sb = const.tile([2, E], bf16, name="stats_sb")
        nc.vector.tensor_copy(stats_sb[:], stats_ps[:])

        # transpose stats to column layout [128, ET, 2] = [cs | u0]
        statsT = const.tile([P, ET, 2], f32, name="statsT")
        for j in range(ET):
            tp = ps_t.tile([P, P], bf16, name="sp")[:, 0:2]
            nc.tensor.matmul(
                tp[:], stats_sb[:, j * P : (j + 1) * P], ident_b[0:2, 0:2],
                is_transpose=True,
            )
            nc.vector.tensor_copy(statsT[:, j, :], tp[:])
        de_col = const.tile([P, ET], f32, name="de_col")
        nc.vector.tensor_scalar(
            out=de_col[:], in0=statsT[:, :, 0], scalar1=EPS, scalar2=None,
            op0=mybir.AluOpType.add,
        )
        nc.vector.reciprocal(de_col[:], de_col[:])

        # ub[e-part, j, bf2] = u0[e] * bias-pattern  (added to every zw m-tile)
        ub = const.tile([P, ET, P], f32, name="ub")
        for j in range(ET):
            nc.vector.tensor_scalar(
                out=ub[:, j, :], in0=bias_bc[:], scalar1=statsT[:, j, 1:2],
                scalar2=None, op0=mybir.AluOpType.mult,
            )

        # Gsd[e, n-col] = de[e] * Gs[n, e] via PE transpose + scaled evict
        # n-col order is (n2, q): col n2*128+q holds n = q*8 + n2
        gsd_all = big.tile([P, ET, N], bf16, name="gsd_all")

        def gsd_strip(k):
            for j in range(ET):
                tp = ps_t.tile([P, P], bf16, name="sp")
                nc.tensor.matmul(
                    tp[:], gs_all[:, k, j * P : (j + 1) * P], ident_b[:],
                    is_transpose=True,
                )
                if (k * ET + j) % 2 == 0:
                    nc.vector.tensor_scalar(
                        out=gsd_all[:, j, k * P : (k + 1) * P], in0=tp[:],
                        scalar1=de_col[:, j : j + 1], scalar2=None,
                        op0=mybir.AluOpType.mult,
                    )
                else:
                    nc.scalar.activation(
                        gsd_all[:, j, k * P : (k + 1) * P], tp[:],
                        mybir.ActivationFunctionType.Copy,
                        scale=de_col[:, j : j + 1],
                    )

        # ---- main pipeline ----------------------------------------------
        # v_all[e-part, j, bf] = zw + u0*bias  (bf16)
        v_all = big.tile([P, ET, BF], bf16, name="v_all")
        os_r = os_.rearrange("b (p n2) f -> p b n2 f", p=P)

        def mm1(m):
            zps = ps_z.tile([P, E], f32, name="zps")
            for k in range(N2):
                nc.tensor.matmul(
                    zps[:], xs_all[:, k, 2 * m : 2 * m + 2, :], gs_all[:, k, :],
                    start=(k == 0), stop=(k == N2 - 1),
                )
            return zps

        def wmm(m, zps):
            zt = ztp.tile([P, E], bf16, name="zt")
            nc.vector.tensor_copy(zt[:], zps[:])
            wps = ps_wo.tile([P, E], f32, name="wps")
            for j in range(ET):
                nc.tensor.matmul(
                    wps[:, j * P : (j + 1) * P], zt[:, j * P : (j + 1) * P], bdw[:],
                    start=True, stop=True,
                )
            # v = ub + zw for all 4 j-blocks in one DVE op
            nc.vector.scalar_tensor_tensor(
                out=v_all[:, :, m * P : (m + 1) * P],
                in0=ub[:],
                scalar=1.0,
                in1=wps[:].rearrange("p (j c) -> p j c", j=ET),
                op0=mybir.AluOpType.mult,
                op1=mybir.AluOpType.add,
            )

        def mm2(c):
            c0, c1 = CHUNKS[c]
            nb = (c1 - c0) // F  # batch entries in this chunk
            ob = osb.tile([P, 8, N2, F], f32, name="ob")
            for k in range(N2):
                ops = ps_wo.tile([P, 512], f32, name="wps")[:, 0 : c1 - c0]
                for j in range(ET):
                    nc.tensor.matmul(
                        ops[:], gsd_all[:, j, k * P : (k + 1) * P],
                        v_all[:, j, c0:c1],
                        start=(j == 0), stop=(j == ET - 1),
                    )
                dst = ob[:, 0:nb, k, :]
                src = ops[:].rearrange("p (c f) -> p c f", f=F)
                if k % 2 == 0:
                    nc.scalar.copy(dst, src)
                else:
                    nc.vector.tensor_copy(dst, src)
            if c >= 3:
                # tail chunks: store each n2-half as soon as its evicts land
                nc.scalar.dma_start(
                    os_r[:, c0 // F : c1 // F, 0:4], ob[:, 0:nb, 0:4, :]
                )
                nc.scalar.dma_start(
                    os_r[:, c0 // F : c1 // F, 4:8], ob[:, 0:nb, 4:8, :]
                )
            else:
                nc.scalar.dma_start(os_r[:, c0 // F : c1 // F], ob[:, 0:nb, :, :])

        # gsd strips are emitted after mm1(1) so the scheduler doesn't place
        # the 32 transposes ahead of MM1(0) -- they're only needed by the
        # first mm2 chunk
        # WMM trails MM1 by TWO m-tiles so the zt evict and v-add never gate
        # the PE even when DVE is mid-way through a long cast/evict
        cast_half(0, 0)
        cast_half(0, 1)
        cast_half(1, 0)
        cast_half(1, 1)
        cast_half(2, 0)
        cast_half(2, 1)
        zq = [mm1(0), mm1(1)]
        for m in range(2, MT):
            if m + 1 < MT:
                cast_half(m + 1, 0)
                cast_half(m + 1, 1)
            zq.append(mm1(m))
            if m == 2:
                for k in range(N2):
                    gsd_strip(k)
            wmm(m - 2, zq.pop(0))
            if m - 2 in CHUNK_AFTER:
                mm2(CHUNK_AFTER[m - 2])
        wmm(MT - 2, zq.pop(0))
        mm2(CHUNK_AFTER[MT - 2])
        wmm(MT - 1, zq.pop(0))
        mm2(CHUNK_AFTER[MT - 1])

    nc.finalize()
    return nc


_NC = None


def _get_nc():
    global _NC
    if _NC is None:
        _NC = _build_nc()
    return _NC


def kernel(x, G, G1, weight, bias):
    nc = _get_nc()
    x = np.ascontiguousarray(x, dtype=np.float32)
    G = np.ascontiguousarray(G, dtype=np.float32)
    G1 = np.ascontiguousarray(G1, dtype=np.float32)
    weight = np.ascontiguousarray(weight, dtype=np.float32)
    bias = np.ascontiguousarray(bias, dtype=np.float32)

    in_maps = []
    for c in range(T):
        in_maps.append(
            {
                "xs": x[c * B : (c + 1) * B],
                "g": G,
                "g1": np.ascontiguousarray(G1[c]),
                "w": weight,
                "b": bias,
            }
        )
    res = bass_utils.run_bass_kernel_spmd(nc, in_maps, core_ids=list(range(T)))
    return np.concatenate([r["os"] for r in res.results], axis=0)


# revision 45
# speedup vs baseline: 1.0538x; 1.0261x over previous
"""HGNN conv kernel for Trainium2, data-parallel over time across 8 cores.

Per core (t = core index): out_b = Dv^-1/2 Gc De^-1 Gc^T Dv^-1/2 (x_b W + 1 b^T)
computed in factored form (L never materialized):
  Gs  = Dv^-1/2 Gc                       [N, E]   (bf16)
  z   = x^T Gs per 128-col bf block      [BF, E]  (MM1)
  zw  = z^T-blocks @ blockdiag(W,W)      [E, BF]  (W-MM; + u0 (x) bias add)
  out = Gsd^T v with Gsd = de * Gs^T     [N, BF]  (MM2)

All matmul operands are bf16 (fp32-family moving operands stream at ~2.4
cycles/col on TRN2 PE vs 1 for bf16).  Nodes are swizzled n = p*8 + n2 so one
SBUF partition covers 8 consecutive DRAM rows -> 2KB-contiguous HBM
descriptors for x loads (f32 staging + permute-cast spread over DVE/ACT/
GpSimd) and for out stores (SBUF staging).  All input DMA rides one HWDGE
FIFO queue (two rings run at ~92 GB/s each vs ~246 for one busy ring):
x0, x1 first (casts overlap the Gc transfer), then Gc, then x2..13.  MM2
runs on bf column chunks interleaved into the m loop so output DMA overlaps
compute; tail chunks shrink so the post-loop drain is short.  Dummy matmuls
at t=0 warm the PE HAM clock to 2.4 GHz.
"""

import sys

import numpy as np

sys.path.insert(0, "/opt/trn_rl_repo")

from contextlib import ExitStack

import concourse.bass as bass
import concourse.mybir as mybir
import concourse.tile as tile
from concourse import bacc, bass_utils
from concourse.masks import make_identity

P = 128
T = 8
B = 28          # batch entries per core
N = 1024        # nodes
E = 512         # hyperedges (256 static + 256 dynamic)
F = 64          # features
BF = B * F      # 1792
EPS = 1e-6
N2 = 8          # node swizzle: n = p*8 + n2
MT = BF // P    # 14 bf-tiles (2 batch entries each)
ET = E // P     # 4 e-tiles
# MM2 output column chunks (bf columns); tail chunks shrink so the last
# MM2+evict+store after the final m-tile is short
CHUNKS = [(0, 512), (512, 1024), (1024, 1408), (1408, 1664), (1664, 1792)]
# emit chunk c right after wmm(m) for m = CHUNK_AFTER[c]
CHUNK_AFTER = {4: 0, 8: 1, 11: 2, 12: 3, 13: 4}

f32 = mybir.dt.float32
f32r = mybir.dt.float32r
bf16 = mybir.dt.bfloat16


def _build_nc():
    nc = bacc.Bacc("TRN2", target_bir_lowering=False, debug=False)

    xs = nc.dram_tensor("xs", [B, N, F], f32, kind="ExternalInput").ap()
    g = nc.dram_tensor("g", [N, 256], f32r, kind="ExternalInput").ap()
    g1 = nc.dram_tensor("g1", [N, 256], f32r, kind="ExternalInput").ap()
    w = nc.dram_tensor("w", [F, F], f32, kind="ExternalInput").ap()
    bvec = nc.dram_tensor("b", [F], f32, kind="ExternalInput").ap()
    os_ = nc.dram_tensor("os", [B, N, F], f32, kind="ExternalOutput").ap()

    with tile.TileContext(nc) as tc, ExitStack() as ctx:
        const = ctx.enter_context(tc.tile_pool(name="const", bufs=1))
        big = ctx.enter_context(tc.tile_pool(name="big", bufs=1))
        xstage = ctx.enter_context(tc.tile_pool(name="xstage", bufs=6))
        ztp = ctx.enter_context(tc.tile_pool(name="ztp", bufs=3))
        osb = ctx.enter_context(tc.tile_pool(name="osb", bufs=2))
        # PSUM: 8 banks, bank-granular per site x bufs:
        # zps 2 + stats 1 + wps(+warm) 1 + ops 2 + transposes 2 = 8
        ps_z = ctx.enter_context(tc.tile_pool(name="ps_z", bufs=2, space="PSUM"))
        ps_st = ctx.enter_context(tc.tile_pool(name="ps_st", bufs=1, space="PSUM"))
        ps_w = ctx.enter_context(tc.tile_pool(name="ps_w", bufs=1, space="PSUM"))
        ps_o = ctx.enter_context(tc.tile_pool(name="ps_o", bufs=2, space="PSUM"))
        ps_t = ctx.enter_context(tc.tile_pool(name="ps_t", bufs=2, space="PSUM"))

        # ---- input DMA: one HWDGE FIFO (sync queue) ----------------------
        bdw_f = const.tile([P, P], f32, name="bdw_f")
        nc.vector.memset(bdw_f[:], 0.0)
        nc.sync.dma_start(bdw_f[0:64, 0:64], w)
        nc.sync.dma_start(bdw_f[64:128, 64:128], w)
        btmp = const.tile([1, F], f32, name="btmp")
        nc.sync.dma_start(btmp[:], bvec[None, :])

        # x staging: [p, b, n2, f] f32, 2KB-contiguous on both sides
        xs_r = xs.rearrange("b (p n2) f -> p b n2 f", p=P)
        xstages = {}

        def stage_x(m):
            xf = xstage.tile([P, 2, N2, F], f32, name="xf")
            nc.sync.dma_start(xf[:], xs_r[:, 2 * m : 2 * m + 2])
            xstages[m] = xf

        stage_x(0)
        stage_x(1)

        # G and G1 in separate [p, n2, 256] tiles with n = p*8 + n2: both
        # sides of each DMA are fully contiguous per partition -> 8KB
        # descriptors; halves so the per-k stats pipeline starts early
        gcg = big.tile([P, N2, 256], f32r, name="gcg")
        gcd = big.tile([P, N2, 256], f32r, name="gcd")
        g_r = g.rearrange("(p n2) e -> p n2 e", p=P)
        g1_r = g1.rearrange("(p n2) e -> p n2 e", p=P)
        for h in range(2):
            nc.sync.dma_start(gcg[:, 4 * h : 4 * h + 4], g_r[:, 4 * h : 4 * h + 4])
            nc.sync.dma_start(gcd[:, 4 * h : 4 * h + 4], g1_r[:, 4 * h : 4 * h + 4])
        for m in range(2, MT):
            stage_x(m)

        # permute-cast staged x -> [p, n2, b, f] bf16 (MM1 lhsT layout),
        # two half-chunks per m spread across DVE / ACT / GpSimd
        xs_all = big.tile([P, N2, B, F], bf16, name="xs_all")

        def cast_half(m, h):
            dst = xs_all[:, 4 * h : 4 * h + 4, 2 * m : 2 * m + 2, :]
            src = xstages[m][:, :, 4 * h : 4 * h + 4, :].rearrange(
                "p b k f -> p k b f"
            )
            if m <= 4:
                nc.vector.tensor_copy(dst, src)
            elif m <= 8:
                nc.scalar.copy(dst, src)
            else:
                nc.gpsimd.tensor_copy(dst, src)

        # ---- constants / PE warmup --------------------------------------
        ident_f = const.tile([P, P], f32, name="ident_f")
        make_identity(nc, ident_f[:])
        ident_b = const.tile([P, P], bf16, name="ident_b")
        nc.vector.tensor_copy(ident_b[:], ident_f[:])

        warm_sb = const.tile([P, 512], bf16, name="warm_sb")
        nc.vector.memset(warm_sb[:], 0.0)
        with tc.high_priority():
            warm_ps = ps_w.tile([P, 512], f32, name="wps")
            for i in range(6):
                nc.tensor.matmul(
                    warm_ps[:], ident_b[:], warm_sb[:], start=(i == 0), stop=(i == 5)
                )

        bdw = const.tile([P, P], bf16, name="bdw")
        nc.vector.tensor_copy(bdw[:], bdw_f[:])

        bias2 = const.tile([1, 2, F], f32, name="bias2")
        nc.vector.tensor_copy(bias2[:], btmp[0:1, None, :].to_broadcast([1, 2, F]))
        bias_bc = const.tile([P, P], f32, name="bias_bc")
        nc.gpsimd.partition_broadcast(
            bias_bc[:], bias2[:].rearrange("o t f -> o (t f)")
        )

        # ---- degree stats, pipelined per n2 ------------------------------
        # dv = 1/sqrt(rowsum(Gc) + eps); gs = dv * Gc (bf16) per k as Gc
        # chunks land
        rsg = const.tile([P, N2], f32, name="rsg")
        rs = const.tile([P, N2], f32, name="rs")
        rs_junk = const.tile([P, 256], f32, name="rs_junk")
        eps_col = const.tile([P, 1], f32, name="eps_col")
        nc.vector.memset(eps_col[:], EPS)
        sq = const.tile([P, N2], f32, name="sq")
        dv = const.tile([P, N2], f32, name="dv")
        gs_all = big.tile([P, N2, E], bf16, name="gs_all")
        # lhsT per n2: [sq | ones] against gs -> colsums of Gc (row 0,
        # since sq*gs = Gc) and of Gs (row 1) in one bf16 matmul chain
        onesq = const.tile([P, N2, 2], bf16, name="onesq")
        nc.vector.memset(onesq[:, :, 1:2], 1.0)
        stats_ps = ps_st.tile([2, E], f32, name="stats_ps")
        for k in range(N2):
            nc.vector.reduce_sum(
                rsg[:, k : k + 1], gcg[:, k, :], axis=mybir.AxisListType.X
            )
            nc.scalar.activation(
                rs_junk[:], gcd[:, k, :],
                mybir.ActivationFunctionType.Copy,
                accum_out=rs[:, k : k + 1],
            )
            nc.vector.scalar_tensor_tensor(
                out=rs[:, k : k + 1], in0=rsg[:, k : k + 1], scalar=1.0,
                in1=rs[:, k : k + 1],
                op0=mybir.AluOpType.mult, op1=mybir.AluOpType.add,
            )
            nc.scalar.activation(
                sq[:, k : k + 1], rs[:, k : k + 1],
                mybir.ActivationFunctionType.Sqrt, bias=eps_col[:],
            )
            nc.vector.reciprocal(dv[:, k : k + 1], sq[:, k : k + 1])
            nc.vector.tensor_copy(onesq[:, k, 0:1], sq[:, k : k + 1])
            nc.vector.tensor_scalar(
                out=gs_all[:, k, 0:256], in0=gcg[:, k, :], scalar1=dv[:, k : k + 1],
                scalar2=None, op0=mybir.AluOpType.mult,
            )
            nc.scalar.activation(
                gs_all[:, k, 256:512], gcd[:, k, :],
                mybir.ActivationFunctionType.Copy, scale=dv[:, k : k + 1],
            )
            nc.tensor.matmul(
                stats_ps[:], onesq[:, k, :], gs_all[:, k, :],
                start=(k == 0), stop=(k == N2 - 1),
            )
        stats_sb = const.tile([2, E], bf16, name="stats_sb")
        nc.vector.tensor_copy(stats_sb[:], stats_ps[:])

        # transpose stats to column layout [128, ET, 2] = [cs | u0]
        statsT = const.tile([P, ET, 2], f32, name="statsT")
        for j in range(ET):
            tp = ps_t.tile([P, P], bf16, name="sp")[:, 0:2]
            nc.tensor.matmul(
                tp[:], stats_sb[:, j * P : (j + 1) * P], ident_b[0:2, 0:2],
                is_transpose=True,
            )
            nc.vector.tensor_copy(statsT[:, j, :], tp[:])
        de_col = const.tile([P, ET], f32, name="de_col")
        nc.vector.tensor_scalar(
            out=de_col[:], in0=statsT[:, :, 0], scalar1=EPS, scalar2=None,
            op0=mybir.AluOpType.add,
        )
        nc.vector.reciprocal(de_col[:], de_col[:])

        # ub[e-part, j, bf2] = u0[e] * bias-pattern  (added to every zw m-tile)
        ub = const.tile([P, ET, P], f32, name="ub")
        for j in range(ET):
            nc.vector.tensor_scalar(
                out=ub[:, j, :], in0=bias_bc[:], scalar1=statsT[:, j, 1:2],
                scalar2=None, op0=mybir.AluOpType.mult,
            )

        # Gsd[e, n-col] = de[e] * Gs[n, e] via PE transpose + scaled evict
        # n-col order is (n2, q): col n2*128+q holds n = q*8 + n2
        gsd_all = big.tile([P, ET, N], bf16, name="gsd_all")

        def gsd_strip(k):
            for j in range(ET):
                tp = ps_t.tile([P, P], bf16, name="sp")
                nc.tensor.matmul(
                    tp[:], gs_all[:, k, j * P : (j + 1) * P], ident_b[:],
                    is_transpose=True,
                )
                if (k * ET + j) % 2 == 0:
                    nc.vector.tensor_scalar(
                        out=gsd_all[:, j, k * P : (k + 1) * P], in0=tp[:],
                        scalar1=de_col[:, j : j + 1], scalar2=None,
                        op0=mybir.AluOpType.mult,
                    )
                else:
                    nc.scalar.activation(
                        gsd_all[:, j, k * P : (k + 1) * P], tp[:],
                        mybir.ActivationFunctionType.Copy,
                        scale=de_col[:, j : j + 1],
                    )

        # ---- main pipeline ----------------------------------------------
        # v_all[e-part, j, bf] = zw + u0*bias  (bf16)
        v_all = big.tile([P, ET, BF], bf16, name="v_all")
        os_r = os_.rearrange("b (p n2) f -> p b n2 f", p=P)

        def mm1(m):
            zps = ps_z.tile([P, E], f32, name="zps")
            for k in range(N2):
                nc.tensor.matmul(
                    zps[:], xs_all[:, k, 2 * m : 2 * m + 2, :], gs_all[:, k, :],
                    start=(k == 0), stop=(k == N2 - 1),
                )
            return zps

        def wmm(m, zps):
            zt = ztp.tile([P, E], bf16, name="zt")
            nc.scalar.copy(zt[:], zps[:])
            wps = ps_w.tile([P, E], f32, name="wps")
            for j in range(ET):
                nc.tensor.matmul(
                    wps[:, j * P : (j + 1) * P], zt[:, j * P : (j + 1) * P], bdw[:],
                    start=True, stop=True,
                )
            # v = ub + zw for all 4 j-blocks in one DVE op
            nc.vector.scalar_tensor_tensor(
                out=v_all[:, :, m * P : (m + 1) * P],
                in0=ub[:],
                scalar=1.0,
                in1=wps[:].rearrange("p (j c) -> p j c", j=ET),
                op0=mybir.AluOpType.mult,
                op1=mybir.AluOpType.add,
            )

        def mm2(c):
            c0, c1 = CHUNKS[c]
            nb = (c1 - c0) // F  # batch entries in this chunk
            ob = osb.tile([P, 8, N2, F], f32, name="ob")
            for k in range(N2):
                ops = ps_o.tile([P, 512], f32, name="ops")[:, 0 : c1 - c0]
                for j in range(ET):
                    nc.tensor.matmul(
                        ops[:], gsd_all[:, j, k * P : (k + 1) * P],
                        v_all[:, j, c0:c1],
                        start=(j == 0), stop=(j == ET - 1),
                    )
                dst = ob[:, 0:nb, k, :]
                src = ops[:].rearrange("p (c f) -> p c f", f=F)
                if k % 2 == 0:
                    nc.scalar.copy(dst, src)
                else:
                    nc.vector.tensor_copy(dst, src)
            if c >= 3:
                # tail chunks: store each n2-half as soon as its evicts land
                nc.scalar.dma_start(
                    os_r[:, c0 // F : c1 // F, 0:4], ob[:, 0:nb, 0:4, :]
                )
                nc.scalar.dma_start(
                    os_r[:, c0 // F : c1 // F, 4:8], ob[:, 0:nb, 4:8, :]
                )
            else:
                nc.scalar.dma_start(os_r[:, c0 // F : c1 // F], ob[:, 0:nb, :, :])

        # gsd strips are emitted after mm1(1) so the scheduler doesn't place
        # the 32 transposes ahead of MM1(0) -- they're only needed by the
        # first mm2 chunk
        cast_half(0, 0)
        cast_half(0, 1)
        cast_half(1, 0)
        cast_half(1, 1)
        zps_prev = mm1(0)
        for m in range(1, MT):
            if m + 1 < MT:
                cast_half(m + 1, 0)
                cast_half(m + 1, 1)
            zps = mm1(m)
            if m == 2:
                for k in range(N2):
                    gsd_strip(k)
            wmm(m - 1, zps_prev)
            zps_prev = zps
            if m - 1 in CHUNK_AFTER:
                mm2(CHUNK_AFTER[m - 1])
        wmm(MT - 1, zps_prev)
        mm2(CHUNK_AFTER[MT - 1])

    nc.finalize()
    return nc


_NC = None


def _get_nc():
    global _NC
    if _NC is None:
        _NC = _build_nc()
    return _NC


def kernel(x, G, G1, weight, bias):
    nc = _get_nc()
    x = np.ascontiguousarray(x, dtype=np.float32)
    G = np.ascontiguousarray(G, dtype=np.float32)
    G1 = np.ascontiguousarray(G1, dtype=np.float32)
    weight = np.ascontiguousarray(weight, dtype=np.float32)
    bias = np.ascontiguousarray(bias, dtype=np.float32)

    in_maps = []
    for c in range(T):
        in_maps.append(
            {
                "xs": x[c * B : (c + 1) * B],
                "g": G,
                "g1": np.ascontiguousarray(G1[c]),
                "w": weight,
                "b": bias,
            }
        )
    res = bass_utils.run_bass_kernel_spmd(nc, in_maps, core_ids=list(range(T)))
    return np.concatenate([r["os"] for r in res.results], axis=0)


# revision 47
# speedup vs baseline: 1.1389x; 1.0808x over previous
"""HGNN conv kernel for Trainium2, data-parallel over time across 8 cores.

Per core (t = core index): out_b = Dv^-1/2 Gc De^-1 Gc^T Dv^-1/2 (x_b W + 1 b^T)
computed in factored form (L never materialized):
  Gs  = Dv^-1/2 Gc                       [N, E]   (bf16)
  z   = x^T Gs per 128-col bf block      [BF, E]  (MM1)
  zw  = z^T-blocks @ blockdiag(W,W)      [E, BF]  (W-MM; + u0 (x) bias add)
  out = Gsd^T v with Gsd = de * Gs^T     [N, BF]  (MM2)

All matmul operands are bf16 (fp32-family moving operands stream at ~2.4
cycles/col on TRN2 PE vs 1 for bf16).  Nodes are swizzled n = p*8 + n2 so one
SBUF partition covers 8 consecutive DRAM rows -> 2KB-contiguous HBM
descriptors for x loads (f32 staging + permute-cast spread over DVE/ACT/
GpSimd) and for out stores (SBUF staging).  All input DMA rides one HWDGE
FIFO queue (two rings run at ~92 GB/s each vs ~246 for one busy ring):
x0, x1 first (casts overlap the Gc transfer), then Gc, then x2..13.  MM2
runs on bf column chunks interleaved into the m loop so output DMA overlaps
compute; tail chunks shrink so the post-loop drain is short.  Dummy matmuls
at t=0 warm the PE HAM clock to 2.4 GHz.
"""

import sys

import numpy as np

sys.path.insert(0, "/opt/trn_rl_repo")

from contextlib import ExitStack

import concourse.bass as bass
import concourse.mybir as mybir
import concourse.tile as tile
from concourse import bacc, bass_utils
from concourse.masks import make_identity

P = 128
T = 8
B = 28          # batch entries per core
N = 1024        # nodes
E = 512         # hyperedges (256 static + 256 dynamic)
F = 64          # features
BF = B * F      # 1792
EPS = 1e-6
N2 = 8          # node swizzle: n = p*8 + n2
MT = BF // P    # 14 bf-tiles (2 batch entries each)
ET = E // P     # 4 e-tiles
# MM2 output column chunks (bf columns); tail chunks shrink so the last
# MM2+evict+store after the final m-tile is short
CHUNKS = [(0, 512), (512, 1024), (1024, 1408), (1408, 1664), (1664, 1792)]
# emit chunk c right after wmm(m) for m = CHUNK_AFTER[c]
CHUNK_AFTER = {4: 0, 8: 1, 11: 2, 12: 3, 13: 4}

f32 = mybir.dt.float32
f32r = mybir.dt.float32r
bf16 = mybir.dt.bfloat16


def _build_nc():
    nc = bacc.Bacc("TRN2", target_bir_lowering=False, debug=False)

    xs = nc.dram_tensor("xs", [B, N, F], f32, kind="ExternalInput").ap()
    g = nc.dram_tensor("g", [N, 256], f32r, kind="ExternalInput").ap()
    g1 = nc.dram_tensor("g1", [N, 256], f32r, kind="ExternalInput").ap()
    w = nc.dram_tensor("w", [F, F], f32, kind="ExternalInput").ap()
    bvec = nc.dram_tensor("b", [F], f32, kind="ExternalInput").ap()
    os_ = nc.dram_tensor("os", [B, N, F], f32, kind="ExternalOutput").ap()

    with tile.TileContext(nc) as tc, ExitStack() as ctx:
        const = ctx.enter_context(tc.tile_pool(name="const", bufs=1))
        big = ctx.enter_context(tc.tile_pool(name="big", bufs=1))
        xstage = ctx.enter_context(tc.tile_pool(name="xstage", bufs=6))
        ztp = ctx.enter_context(tc.tile_pool(name="ztp", bufs=3))
        osb = ctx.enter_context(tc.tile_pool(name="osb", bufs=2))
        # PSUM: 8 banks, bank-granular per site x bufs:
        # zps 2 + stats 1 + wps(+warm) 1 + ops 2 + transposes 2 = 8
        ps_z = ctx.enter_context(tc.tile_pool(name="ps_z", bufs=2, space="PSUM"))
        ps_st = ctx.enter_context(tc.tile_pool(name="ps_st", bufs=1, space="PSUM"))
        ps_w = ctx.enter_context(tc.tile_pool(name="ps_w", bufs=1, space="PSUM"))
        ps_o = ctx.enter_context(tc.tile_pool(name="ps_o", bufs=2, space="PSUM"))
        ps_t = ctx.enter_context(tc.tile_pool(name="ps_t", bufs=2, space="PSUM"))

        # ---- input DMA: one HWDGE FIFO (sync queue) ----------------------
        bdw_f = const.tile([P, P], f32, name="bdw_f")
        nc.vector.memset(bdw_f[:], 0.0)
        nc.sync.dma_start(bdw_f[0:64, 0:64], w)
        nc.sync.dma_start(bdw_f[64:128, 64:128], w)
        btmp = const.tile([1, F], f32, name="btmp")
        nc.sync.dma_start(btmp[:], bvec[None, :])

        # x staging: [p, b, n2, f] f32, 2KB-contiguous on both sides
        xs_r = xs.rearrange("b (p n2) f -> p b n2 f", p=P)
        xstages = {}

        def stage_x(m):
            xf = xstage.tile([P, 2, N2, F], f32, name="xf")
            nc.sync.dma_start(xf[:], xs_r[:, 2 * m : 2 * m + 2])
            xstages[m] = xf

        # G and G1 in separate [p, n2, 256] tiles with n = p*8 + n2: both
        # sides of each DMA are fully contiguous per partition -> 8KB
        # descriptors; halves so the per-k stats pipeline starts early.
        # Gc strictly first: it gates the gs chain which gates MM1.
        gcg = big.tile([P, N2, 256], f32r, name="gcg")
        gcd = big.tile([P, N2, 256], f32r, name="gcd")
        g_r = g.rearrange("(p n2) e -> p n2 e", p=P)
        g1_r = g1.rearrange("(p n2) e -> p n2 e", p=P)
        for h in range(2):
            nc.sync.dma_start(gcg[:, 4 * h : 4 * h + 4], g_r[:, 4 * h : 4 * h + 4])
            nc.sync.dma_start(gcd[:, 4 * h : 4 * h + 4], g1_r[:, 4 * h : 4 * h + 4])
        for m in range(MT):
            stage_x(m)

        # permute-cast staged x -> [p, n2, b, f] bf16 (MM1 lhsT layout),
        # two half-chunks per m spread across DVE / ACT / GpSimd
        xs_all = big.tile([P, N2, B, F], bf16, name="xs_all")

        def cast_half(m, h):
            dst = xs_all[:, 4 * h : 4 * h + 4, 2 * m : 2 * m + 2, :]
            src = xstages[m][:, :, 4 * h : 4 * h + 4, :].rearrange(
                "p b k f -> p k b f"
            )
            if m <= 4:
                nc.vector.tensor_copy(dst, src)
            elif m <= 8:
                nc.scalar.copy(dst, src)
            else:
                nc.gpsimd.tensor_copy(dst, src)

        # ---- constants / PE warmup --------------------------------------
        ident_f = const.tile([P, P], f32, name="ident_f")
        make_identity(nc, ident_f[:])
        ident_b = const.tile([P, P], bf16, name="ident_b")
        nc.vector.tensor_copy(ident_b[:], ident_f[:])

        warm_sb = const.tile([P, 512], bf16, name="warm_sb")
        nc.vector.memset(warm_sb[:], 0.0)
        with tc.high_priority():
            warm_ps = ps_w.tile([P, 512], f32, name="wps")
            for i in range(8):
                nc.tensor.matmul(
                    warm_ps[:], ident_b[:], warm_sb[:], start=(i == 0), stop=(i == 7)
                )

        bdw = const.tile([P, P], bf16, name="bdw")
        nc.vector.tensor_copy(bdw[:], bdw_f[:])

        bias2 = const.tile([1, 2, F], f32, name="bias2")
        nc.vector.tensor_copy(bias2[:], btmp[0:1, None, :].to_broadcast([1, 2, F]))
        bias_bc = const.tile([P, P], f32, name="bias_bc")
        nc.gpsimd.partition_broadcast(
            bias_bc[:], bias2[:].rearrange("o t f -> o (t f)")
        )

        # ---- degree stats, pipelined per n2 ------------------------------
        # dv = 1/sqrt(rowsum(Gc) + eps); gs = dv * Gc (bf16) per k as Gc
        # chunks land
        rsg = const.tile([P, N2], f32, name="rsg")
        rs = const.tile([P, N2], f32, name="rs")
        rs_junk = const.tile([P, 256], f32, name="rs_junk")
        eps_col = const.tile([P, 1], f32, name="eps_col")
        nc.vector.memset(eps_col[:], EPS)
        sq = const.tile([P, N2], f32, name="sq")
        dv = const.tile([P, N2], f32, name="dv")
        gs_all = big.tile([P, N2, E], bf16, name="gs_all")
        # lhsT per n2: [sq | ones] against gs -> colsums of Gc (row 0,
        # since sq*gs = Gc) and of Gs (row 1) in one bf16 matmul chain
        onesq = const.tile([P, N2, 2], bf16, name="onesq")
        nc.vector.memset(onesq[:, :, 1:2], 1.0)
        stats_ps = ps_st.tile([2, E], f32, name="stats_ps")
        for k in range(N2):
            nc.vector.reduce_sum(
                rsg[:, k : k + 1], gcg[:, k, :], axis=mybir.AxisListType.X
            )
            nc.scalar.activation(
                rs_junk[:], gcd[:, k, :],
                mybir.ActivationFunctionType.Copy,
                accum_out=rs[:, k : k + 1],
            )
            nc.vector.scalar_tensor_tensor(
                out=rs[:, k : k + 1], in0=rsg[:, k : k + 1], scalar=1.0,
                in1=rs[:, k : k + 1],
                op0=mybir.AluOpType.mult, op1=mybir.AluOpType.add,
            )
            nc.scalar.activation(
                sq[:, k : k + 1], rs[:, k : k + 1],
                mybir.ActivationFunctionType.Sqrt, bias=eps_col[:],
            )
            nc.vector.reciprocal(dv[:, k : k + 1], sq[:, k : k + 1])
            nc.vector.tensor_copy(onesq[:, k, 0:1], sq[:, k : k + 1])
            nc.vector.tensor_scalar(
                out=gs_all[:, k, 0:256], in0=gcg[:, k, :], scalar1=dv[:, k : k + 1],
                scalar2=None, op0=mybir.AluOpType.mult,
            )
            nc.scalar.activation(
                gs_all[:, k, 256:512], gcd[:, k, :],
                mybir.ActivationFunctionType.Copy, scale=dv[:, k : k + 1],
            )
            nc.tensor.matmul(
                stats_ps[:], onesq[:, k, :], gs_all[:, k, :],
                start=(k == 0), stop=(k == N2 - 1),
            )
        stats_sb = const.tile([2, E], bf16, name="stats_sb")
        nc.vector.tensor_copy(stats_sb[:], stats_ps[:])

        # transpose stats to column layout [128, ET, 2] = [cs | u0]
        statsT = const.tile([P, ET, 2], f32, name="statsT")
        for j in range(ET):
            tp = ps_t.tile([P, P], bf16, name="sp")[:, 0:2]
            nc.tensor.matmul(
                tp[:], stats_sb[:, j * P : (j + 1) * P], ident_b[0:2, 0:2],
                is_transpose=True,
            )
            nc.vector.tensor_copy(statsT[:, j, :], tp[:])
        de_col = const.tile([P, ET], f32, name="de_col")
        nc.vector.tensor_scalar(
            out=de_col[:], in0=statsT[:, :, 0], scalar1=EPS, scalar2=None,
            op0=mybir.AluOpType.add,
        )
        nc.vector.reciprocal(de_col[:], de_col[:])

        # ub[e-part, j, bf2] = u0[e] * bias-pattern  (added to every zw m-tile)
        ub = const.tile([P, ET, P], f32, name="ub")
        for j in range(ET):
            nc.vector.tensor_scalar(
                out=ub[:, j, :], in0=bias_bc[:], scalar1=statsT[:, j, 1:2],
                scalar2=None, op0=mybir.AluOpType.mult,
            )

        # Gsd[e, n-col] = de[e] * Gs[n, e] via PE transpose + scaled evict
        # n-col order is (n2, q): col n2*128+q holds n = q*8 + n2
        gsd_all = big.tile([P, ET, N], bf16, name="gsd_all")

        def gsd_strip(k):
            for j in range(ET):
                tp = ps_t.tile([P, P], bf16, name="sp")
                nc.tensor.matmul(
                    tp[:], gs_all[:, k, j * P : (j + 1) * P], ident_b[:],
                    is_transpose=True,
                )
                if (k * ET + j) % 2 == 0:
                    nc.vector.tensor_scalar(
                        out=gsd_all[:, j, k * P : (k + 1) * P], in0=tp[:],
                        scalar1=de_col[:, j : j + 1], scalar2=None,
                        op0=mybir.AluOpType.mult,
                    )
                else:
                    nc.scalar.activation(
                        gsd_all[:, j, k * P : (k + 1) * P], tp[:],
                        mybir.ActivationFunctionType.Copy,
                        scale=de_col[:, j : j + 1],
                    )

        # ---- main pipeline ----------------------------------------------
        # v_all[e-part, j, bf] = zw + u0*bias  (bf16)
        v_all = big.tile([P, ET, BF], bf16, name="v_all")
        os_r = os_.rearrange("b (p n2) f -> p b n2 f", p=P)

        def mm1(m):
            zps = ps_z.tile([P, E], f32, name="zps")
            for k in range(N2):
                nc.tensor.matmul(
                    zps[:], xs_all[:, k, 2 * m : 2 * m + 2, :], gs_all[:, k, :],
                    start=(k == 0), stop=(k == N2 - 1),
                )
            return zps

        def wmm(m, zps):
            zt = ztp.tile([P, E], bf16, name="zt")
            nc.scalar.copy(zt[:], zps[:])
            wps = ps_w.tile([P, E], f32, name="wps")
            for j in range(ET):
                nc.tensor.matmul(
                    wps[:, j * P : (j + 1) * P], zt[:, j * P : (j + 1) * P], bdw[:],
                    start=True, stop=True,
                )
            # v = ub + zw for all 4 j-blocks in one DVE op
            nc.vector.scalar_tensor_tensor(
                out=v_all[:, :, m * P : (m + 1) * P],
                in0=ub[:],
                scalar=1.0,
                in1=wps[:].rearrange("p (j c) -> p j c", j=ET),
                op0=mybir.AluOpType.mult,
                op1=mybir.AluOpType.add,
            )

        def mm2(c):
            c0, c1 = CHUNKS[c]
            nb = (c1 - c0) // F  # batch entries in this chunk
            ob = osb.tile([P, 8, N2, F], f32, name="ob")
            for k in range(N2):
                ops = ps_o.tile([P, 512], f32, name="ops")[:, 0 : c1 - c0]
                for j in range(ET):
                    nc.tensor.matmul(
                        ops[:], gsd_all[:, j, k * P : (k + 1) * P],
                        v_all[:, j, c0:c1],
                        start=(j == 0), stop=(j == ET - 1),
                    )
                dst = ob[:, 0:nb, k, :]
                src = ops[:].rearrange("p (c f) -> p c f", f=F)
                if k % 2 == 0:
                    nc.scalar.copy(dst, src)
                else:
                    nc.vector.tensor_copy(dst, src)
            if c >= 3:
                # tail chunks: store each n2-half as soon as its evicts land
                nc.scalar.dma_start(
                    os_r[:, c0 // F : c1 // F, 0:4], ob[:, 0:nb, 0:4, :]
                )
                nc.scalar.dma_start(
                    os_r[:, c0 // F : c1 // F, 4:8], ob[:, 0:nb, 4:8, :]
                )
            else:
                nc.scalar.dma_start(os_r[:, c0 // F : c1 // F], ob[:, 0:nb, :, :])

        # gsd strips are emitted after mm1(1) so the scheduler doesn't place
        # the 32 transposes ahead of MM1(0) -- they're only needed by the
        # first mm2 chunk
        cast_half(0, 0)
        cast_half(0, 1)
        cast_half(1, 0)
        cast_half(1, 1)
        zps_prev = mm1(0)
        for m in range(1, MT):
            if m + 1 < MT:
                cast_half(m + 1, 0)
                cast_half(m + 1, 1)
            zps = mm1(m)
            if m == 2:
                for k in range(N2):
                    gsd_strip(k)
            wmm(m - 1, zps_prev)
            zps_prev = zps
            if m - 1 in CHUNK_AFTER:
                mm2(CHUNK_AFTER[m - 1])
        wmm(MT - 1, zps_prev)
        mm2(CHUNK_AFTER[MT - 1])

    nc.finalize()
    return nc


_NC = None


def _get_nc():
    global _NC
    if _NC is None:
        _NC = _build_nc()
    return _NC


def kernel(x, G, G1, weight, bias):
    nc = _get_nc()
    x = np.ascontiguousarray(x, dtype=np.float32)
    G = np.ascontiguousarray(G, dtype=np.float32)
    G1 = np.ascontiguousarray(G1, dtype=np.float32)
    weight = np.ascontiguousarray(weight, dtype=np.float32)
    bias = np.ascontiguousarray(bias, dtype=np.float32)

    in_maps = []
    for c in range(T):
        in_maps.append(
            {
                "xs": x[c * B : (c + 1) * B],
                "g": G,
                "g1": np.ascontiguousarray(G1[c]),
                "w": weight,
                "b": bias,
            }
        )
    res = bass_utils.run_bass_kernel_spmd(nc, in_maps, core_ids=list(range(T)))
    return np.concatenate([r["os"] for r in res.results], axis=0)


# revision 48
# speedup vs baseline: 1.1437x; 1.0042x over previous
"""HGNN conv kernel for Trainium2, data-parallel over time across 8 cores.

Per core (t = core index): out_b = Dv^-1/2 Gc De^-1 Gc^T Dv^-1/2 (x_b W + 1 b^T)
computed in factored form (L never materialized):
  Gs  = Dv^-1/2 Gc                       [N, E]   (bf16)
  z   = x^T Gs per 128-col bf block      [BF, E]  (MM1)
  zw  = z^T-blocks @ blockdiag(W,W)      [E, BF]  (W-MM; + u0 (x) bias add)
  out = Gsd^T v with Gsd = de * Gs^T     [N, BF]  (MM2)

All matmul operands are bf16 (fp32-family moving operands stream at ~2.4
cycles/col on TRN2 PE vs 1 for bf16).  Nodes are swizzled n = p*8 + n2 so one
SBUF partition covers 8 consecutive DRAM rows -> 2KB-contiguous HBM
descriptors for x loads (f32 staging + permute-cast spread over DVE/ACT/
GpSimd) and for out stores (SBUF staging).  All input DMA rides one HWDGE
FIFO queue (two rings run at ~92 GB/s each vs ~246 for one busy ring):
x0, x1 first (casts overlap the Gc transfer), then Gc, then x2..13.  MM2
runs on bf column chunks interleaved into the m loop so output DMA overlaps
compute; tail chunks shrink so the post-loop drain is short.  Dummy matmuls
at t=0 warm the PE HAM clock to 2.4 GHz.
"""

import sys

import numpy as np

sys.path.insert(0, "/opt/trn_rl_repo")

from contextlib import ExitStack

import concourse.bass as bass
import concourse.mybir as mybir
import concourse.tile as tile
from concourse import bacc, bass_utils
from concourse.masks import make_identity

P = 128
T = 8
B = 28          # batch entries per core
N = 1024        # nodes
E = 512         # hyperedges (256 static + 256 dynamic)
F = 64          # features
BF = B * F      # 1792
EPS = 1e-6
N2 = 8          # node swizzle: n = p*8 + n2
MT = BF // P    # 14 bf-tiles (2 batch entries each)
ET = E // P     # 4 e-tiles
# MM2 output column chunks (bf columns); tail chunks shrink so the last
# MM2+evict+store after the final m-tile is short
CHUNKS = [(0, 512), (512, 1024), (1024, 1408), (1408, 1664), (1664, 1792)]
# emit chunk c right after wmm(m) for m = CHUNK_AFTER[c]
CHUNK_AFTER = {4: 0, 8: 1, 11: 2, 12: 3, 13: 4}

f32 = mybir.dt.float32
f32r = mybir.dt.float32r
bf16 = mybir.dt.bfloat16


def _build_nc():
    nc = bacc.Bacc("TRN2", target_bir_lowering=False, debug=False)

    xs = nc.dram_tensor("xs", [B, N, F], f32, kind="ExternalInput").ap()
    g = nc.dram_tensor("g", [N, 256], f32r, kind="ExternalInput").ap()
    g1 = nc.dram_tensor("g1", [N, 256], f32r, kind="ExternalInput").ap()
    w = nc.dram_tensor("w", [F, F], f32, kind="ExternalInput").ap()
    bvec = nc.dram_tensor("b", [F], f32, kind="ExternalInput").ap()
    os_ = nc.dram_tensor("os", [B, N, F], f32, kind="ExternalOutput").ap()

    with tile.TileContext(nc) as tc, ExitStack() as ctx:
        const = ctx.enter_context(tc.tile_pool(name="const", bufs=1))
        big = ctx.enter_context(tc.tile_pool(name="big", bufs=1))
        xstage = ctx.enter_context(tc.tile_pool(name="xstage", bufs=6))
        ztp = ctx.enter_context(tc.tile_pool(name="ztp", bufs=3))
        osb = ctx.enter_context(tc.tile_pool(name="osb", bufs=2))
        # PSUM: 8 banks, bank-granular per site x bufs:
        # zps 2 + stats 1 + wps(+warm) 1 + ops 2 + transposes 2 = 8
        ps_z = ctx.enter_context(tc.tile_pool(name="ps_z", bufs=2, space="PSUM"))
        ps_st = ctx.enter_context(tc.tile_pool(name="ps_st", bufs=1, space="PSUM"))
        ps_w = ctx.enter_context(tc.tile_pool(name="ps_w", bufs=1, space="PSUM"))
        ps_o = ctx.enter_context(tc.tile_pool(name="ps_o", bufs=2, space="PSUM"))
        ps_t = ctx.enter_context(tc.tile_pool(name="ps_t", bufs=2, space="PSUM"))

        # ---- input DMA: one HWDGE FIFO (sync queue) ----------------------
        bdw_f = const.tile([P, P], f32, name="bdw_f")
        nc.vector.memset(bdw_f[:], 0.0)
        nc.sync.dma_start(bdw_f[0:64, 0:64], w)
        nc.sync.dma_start(bdw_f[64:128, 64:128], w)
        btmp = const.tile([1, F], f32, name="btmp")
        nc.sync.dma_start(btmp[:], bvec[None, :])

        # x staging: [p, b, n2, f] f32, 2KB-contiguous on both sides
        xs_r = xs.rearrange("b (p n2) f -> p b n2 f", p=P)
        xstages = {}

        def stage_x(m):
            xf = xstage.tile([P, 2, N2, F], f32, name="xf")
            nc.sync.dma_start(xf[:], xs_r[:, 2 * m : 2 * m + 2])
            xstages[m] = xf

        # G and G1 in separate [p, n2, 256] tiles with n = p*8 + n2: both
        # sides of each DMA are fully contiguous per partition -> 8KB
        # descriptors; halves so the per-k stats pipeline starts early.
        # Gc strictly first: it gates the gs chain which gates MM1.
        gcg = big.tile([P, N2, 256], f32r, name="gcg")
        gcd = big.tile([P, N2, 256], f32r, name="gcd")
        g_r = g.rearrange("(p n2) e -> p n2 e", p=P)
        g1_r = g1.rearrange("(p n2) e -> p n2 e", p=P)
        for h in range(2):
            nc.sync.dma_start(gcg[:, 4 * h : 4 * h + 4], g_r[:, 4 * h : 4 * h + 4])
            nc.sync.dma_start(gcd[:, 4 * h : 4 * h + 4], g1_r[:, 4 * h : 4 * h + 4])
        for m in range(MT):
            stage_x(m)

        # permute-cast staged x -> [p, n2, b, f] bf16 (MM1 lhsT layout),
        # two half-chunks per m spread across DVE / ACT / GpSimd
        xs_all = big.tile([P, N2, B, F], bf16, name="xs_all")

        def cast_half(m, h):
            dst = xs_all[:, 4 * h : 4 * h + 4, 2 * m : 2 * m + 2, :]
            src = xstages[m][:, :, 4 * h : 4 * h + 4, :].rearrange(
                "p b k f -> p k b f"
            )
            if m <= 4:
                nc.vector.tensor_copy(dst, src)
            elif m <= 8:
                nc.scalar.copy(dst, src)
            else:
                nc.gpsimd.tensor_copy(dst, src)

        # ---- constants / PE warmup --------------------------------------
        ident_f = const.tile([P, P], f32, name="ident_f")
        make_identity(nc, ident_f[:])
        ident_b = const.tile([P, P], bf16, name="ident_b")
        nc.vector.tensor_copy(ident_b[:], ident_f[:])

        # gate warmup on the last Gc chunk: runs ~15-17us, right before the
        # dense phase, so HAM doesn't re-throttle in between (an early warmup
        # expires during the input wait)
        warm_sb = const.tile([P, 512], bf16, name="warm_sb")
        nc.vector.tensor_copy(warm_sb[:], gcd[:, 6:8, :])
        with tc.high_priority():
            warm_ps = ps_w.tile([P, 512], f32, name="wps")
            for i in range(8):
                nc.tensor.matmul(
                    warm_ps[:], ident_b[:], warm_sb[:], start=(i == 0), stop=(i == 7)
                )

        bdw = const.tile([P, P], bf16, name="bdw")
        nc.vector.tensor_copy(bdw[:], bdw_f[:])

        bias2 = const.tile([1, 2, F], f32, name="bias2")
        nc.vector.tensor_copy(bias2[:], btmp[0:1, None, :].to_broadcast([1, 2, F]))
        bias_bc = const.tile([P, P], f32, name="bias_bc")
        nc.gpsimd.partition_broadcast(
            bias_bc[:], bias2[:].rearrange("o t f -> o (t f)")
        )

        # ---- degree stats, pipelined per n2 ------------------------------
        # dv = 1/sqrt(rowsum(Gc) + eps); gs = dv * Gc (bf16) per k as Gc
        # chunks land
        rsg = const.tile([P, N2], f32, name="rsg")
        rs = const.tile([P, N2], f32, name="rs")
        rs_junk = const.tile([P, 256], f32, name="rs_junk")
        eps_col = const.tile([P, 1], f32, name="eps_col")
        nc.vector.memset(eps_col[:], EPS)
        sq = const.tile([P, N2], f32, name="sq")
        dv = const.tile([P, N2], f32, name="dv")
        gs_all = big.tile([P, N2, E], bf16, name="gs_all")
        # lhsT per n2: [sq | ones] against gs -> colsums of Gc (row 0,
        # since sq*gs = Gc) and of Gs (row 1) in one bf16 matmul chain
        onesq = const.tile([P, N2, 2], bf16, name="onesq")
        nc.vector.memset(onesq[:, :, 1:2], 1.0)
        stats_ps = ps_st.tile([2, E], f32, name="stats_ps")
        for k in range(N2):
            nc.vector.reduce_sum(
                rsg[:, k : k + 1], gcg[:, k, :], axis=mybir.AxisListType.X
            )
            nc.scalar.activation(
                rs_junk[:], gcd[:, k, :],
                mybir.ActivationFunctionType.Copy,
                accum_out=rs[:, k : k + 1],
            )
            nc.vector.scalar_tensor_tensor(
                out=rs[:, k : k + 1], in0=rsg[:, k : k + 1], scalar=1.0,
                in1=rs[:, k : k + 1],
                op0=mybir.AluOpType.mult, op1=mybir.AluOpType.add,
            )
            nc.scalar.activation(
                sq[:, k : k + 1], rs[:, k : k + 1],
                mybir.ActivationFunctionType.Sqrt, bias=eps_col[:],
            )
            nc.vector.reciprocal(dv[:, k : k + 1], sq[:, k : k + 1])
            nc.vector.tensor_copy(onesq[:, k, 0:1], sq[:, k : k + 1])
            nc.vector.tensor_scalar(
                out=gs_all[:, k, 0:256], in0=gcg[:, k, :], scalar1=dv[:, k : k + 1],
                scalar2=None, op0=mybir.AluOpType.mult,
            )
            nc.scalar.activation(
                gs_all[:, k, 256:512], gcd[:, k, :],
                mybir.ActivationFunctionType.Copy, scale=dv[:, k : k + 1],
            )
            nc.tensor.matmul(
                stats_ps[:], onesq[:, k, :], gs_all[:, k, :],
                start=(k == 0), stop=(k == N2 - 1),
            )
        stats_sb = const.tile([2, E], bf16, name="stats_sb")
        nc.vector.tensor_copy(stats_sb[:], stats_ps[:])

        # transpose stats to column layout [128, ET, 2] = [cs | u0]
        statsT = const.tile([P, ET, 2], f32, name="statsT")
        for j in range(ET):
            tp = ps_t.tile([P, P], bf16, name="sp")[:, 0:2]
            nc.tensor.matmul(
                tp[:], stats_sb[:, j * P : (j + 1) * P], ident_b[0:2, 0:2],
                is_transpose=True,
            )
            nc.vector.tensor_copy(statsT[:, j, :], tp[:])
        de_col = const.tile([P, ET], f32, name="de_col")
        nc.vector.tensor_scalar(
            out=de_col[:], in0=statsT[:, :, 0], scalar1=EPS, scalar2=None,
            op0=mybir.AluOpType.add,
        )
        nc.vector.reciprocal(de_col[:], de_col[:])

        # ub[e-part, j, bf2] = u0[e] * bias-pattern  (added to every zw m-tile)
        ub = const.tile([P, ET, P], f32, name="ub")
        for j in range(ET):
            nc.vector.tensor_scalar(
                out=ub[:, j, :], in0=bias_bc[:], scalar1=statsT[:, j, 1:2],
                scalar2=None, op0=mybir.AluOpType.mult,
            )

        # Gsd[e, n-col] = de[e] * Gs[n, e] via PE transpose + scaled evict
        # n-col order is (n2, q): col n2*128+q holds n = q*8 + n2
        gsd_all = big.tile([P, ET, N], bf16, name="gsd_all")

        def gsd_strip(k):
            for j in range(ET):
                tp = ps_t.tile([P, P], bf16, name="sp")
                nc.tensor.matmul(
                    tp[:], gs_all[:, k, j * P : (j + 1) * P], ident_b[:],
                    is_transpose=True,
                )
                if (k * ET + j) % 2 == 0:
                    nc.vector.tensor_scalar(
                        out=gsd_all[:, j, k * P : (k + 1) * P], in0=tp[:],
                        scalar1=de_col[:, j : j + 1], scalar2=None,
                        op0=mybir.AluOpType.mult,
                    )
                else:
                    nc.scalar.activation(
                        gsd_all[:, j, k * P : (k + 1) * P], tp[:],
                        mybir.ActivationFunctionType.Copy,
                        scale=de_col[:, j : j + 1],
                    )

        # ---- main pipeline ----------------------------------------------
        # v_all[e-part, j, bf] = zw + u0*bias  (bf16)
        v_all = big.tile([P, ET, BF], bf16, name="v_all")
        os_r = os_.rearrange("b (p n2) f -> p b n2 f", p=P)

        def mm1(m):
            zps = ps_z.tile([P, E], f32, name="zps")
            for k in range(N2):
                nc.tensor.matmul(
                    zps[:], xs_all[:, k, 2 * m : 2 * m + 2, :], gs_all[:, k, :],
                    start=(k == 0), stop=(k == N2 - 1),
                )
            return zps

        def wmm(m, zps):
            zt = ztp.tile([P, E], bf16, name="zt")
            nc.scalar.copy(zt[:], zps[:])
            wps = ps_w.tile([P, E], f32, name="wps")
            for j in range(ET):
                nc.tensor.matmul(
                    wps[:, j * P : (j + 1) * P], zt[:, j * P : (j + 1) * P], bdw[:],
                    start=True, stop=True,
                )
            # v = ub + zw for all 4 j-blocks in one DVE op
            nc.vector.scalar_tensor_tensor(
                out=v_all[:, :, m * P : (m + 1) * P],
                in0=ub[:],
                scalar=1.0,
                in1=wps[:].rearrange("p (j c) -> p j c", j=ET),
                op0=mybir.AluOpType.mult,
                op1=mybir.AluOpType.add,
            )

        def mm2(c):
            c0, c1 = CHUNKS[c]
            nb = (c1 - c0) // F  # batch entries in this chunk
            ob = osb.tile([P, 8, N2, F], f32, name="ob")
            for k in range(N2):
                ops = ps_o.tile([P, 512], f32, name="ops")[:, 0 : c1 - c0]
                for j in range(ET):
                    nc.tensor.matmul(
                        ops[:], gsd_all[:, j, k * P : (k + 1) * P],
                        v_all[:, j, c0:c1],
                        start=(j == 0), stop=(j == ET - 1),
                    )
                dst = ob[:, 0:nb, k, :]
                src = ops[:].rearrange("p (c f) -> p c f", f=F)
                if k % 2 == 0:
                    nc.scalar.copy(dst, src)
                else:
                    nc.vector.tensor_copy(dst, src)
            if c >= 3:
                # tail chunks: store each n2-half as soon as its evicts land
                nc.scalar.dma_start(
                    os_r[:, c0 // F : c1 // F, 0:4], ob[:, 0:nb, 0:4, :]
                )
                nc.scalar.dma_start(
                    os_r[:, c0 // F : c1 // F, 4:8], ob[:, 0:nb, 4:8, :]
                )
            else:
                nc.scalar.dma_start(os_r[:, c0 // F : c1 // F], ob[:, 0:nb, :, :])

        # gsd strips are emitted after mm1(1) so the scheduler doesn't place
        # the 32 transposes ahead of MM1(0) -- they're only needed by the
        # first mm2 chunk
        cast_half(0, 0)
        cast_half(0, 1)
        cast_half(1, 0)
        cast_half(1, 1)
        zps_prev = mm1(0)
        for m in range(1, MT):
            if m + 1 < MT:
                cast_half(m + 1, 0)
                cast_half(m + 1, 1)
            zps = mm1(m)
            if m == 2:
                for k in range(N2):
                    gsd_strip(k)
            wmm(m - 1, zps_prev)
            zps_prev = zps
            if m - 1 in CHUNK_AFTER:
                mm2(CHUNK_AFTER[m - 1])
        wmm(MT - 1, zps_prev)
        mm2(CHUNK_AFTER[MT - 1])

    nc.finalize()
    return nc


_NC = None


def _get_nc():
    global _NC
    if _NC is None:
        _NC = _build_nc()
    return _NC


def kernel(x, G, G1, weight, bias):
    nc = _get_nc()
    x = np.ascontiguousarray(x, dtype=np.float32)
    G = np.ascontiguousarray(G, dtype=np.float32)
    G1 = np.ascontiguousarray(G1, dtype=np.float32)
    weight = np.ascontiguousarray(weight, dtype=np.float32)
    bias = np.ascontiguousarray(bias, dtype=np.float32)

    in_maps = []
    for c in range(T):
        in_maps.append(
            {
                "xs": x[c * B : (c + 1) * B],
                "g": G,
                "g1": np.ascontiguousarray(G1[c]),
                "w": weight,
                "b": bias,
            }
        )
    res = bass_utils.run_bass_kernel_spmd(nc, in_maps, core_ids=list(range(T)))
    return np.concatenate([r["os"] for r in res.results], axis=0)
